# revision 10
# baseline (speedup 1.0000x reference)
"""Trainium2 Bass kernel for a capsule-network (MIND-style) interest extractor.

Math (per batch element b):
  hat[b,s,(n,d')] = sum_d u[b,s,d] * w[s, n*D+d', d]          (bilinear map)
  3 rounds of dynamic routing over s (softmax across n, masked), then
  interest_capsule = squash(sum_s sw*hat), readout = capsule[argmax_n <cap_n, eb>].

Distribution: pure data-parallel over batch (B=2048 -> 8 cores x 256), w replicated.

Per-core device algorithm (fp32 throughout; per b-tile of 128 rows):
  - u is host-transposed/packed to uT2[(q,d), j, b]  (s = 2j+q) so the
    per-s matmul lhsT/rhs slices need no on-chip transposes.
  - hat is never materialized in SBUF (too big); it is recomputed on the
    TensorEngine into PSUM in s-batches for each of the 5 routing passes
    (E1-iter0, delta0, E1-iter1, delta1, E1-iter2), and consumed by
    VectorEngine ops directly out of PSUM:
      E1 pass:    tmp = sw (x) hat ; cap_acc += reduce_s(tmp)
      delta pass: tmp = cap (x) hat ; delta = reduce_d'(tmp)
  - softmax over n (4), squash, and the hard readout (argmax over 4) are
    small per-(b) vector ops.
"""

import sys

import numpy as np

for _p in ("/opt/trn_rl_repo",):
    if _p not in sys.path:
        sys.path.insert(0, _p)

B, S, D, NI = 2048, 200, 64, 4
K = NI * D          # 256
NCORES = 8
BL = B // NCORES    # 256 rows per core
P = 128             # partitions / b-tile rows
NT = BL // P        # b-tiles per core (2)
J = S // 2          # s-pairs (100)
SB = 8              # s-values per PSUM batch
NSB = S // SB       # batches per hat pass (25)

_f32 = None  # set lazily (mybir.dt.float32)


def _build_bass():
    from contextlib import ExitStack

    import concourse.bacc as bacc
    import concourse.tile as tile
    from concourse import mybir

    f32 = mybir.dt.float32
    Alu = mybir.AluOpType
    Act = mybir.ActivationFunctionType

    nc = bacc.Bacc("TRN2", target_bir_lowering=False)

    uT2 = nc.declare_dram_parameter("uT2", [P, J, BL], f32, isOutput=False)
    wT2 = nc.declare_dram_parameter("wT2", [P, J, K], f32, isOutput=False)
    mask_d = nc.declare_dram_parameter("mask", [BL, S], f32, isOutput=False)
    eb_d = nc.declare_dram_parameter("eb", [BL, D], f32, isOutput=False)
    cap_d = nc.declare_dram_parameter("cap", [BL, K], f32, isOutput=True)
    ro_d = nc.declare_dram_parameter("ro", [BL, D], f32, isOutput=True)

    with tile.TileContext(nc) as tc, ExitStack() as ctx:
        singles = ctx.enter_context(tc.tile_pool(name="singles", bufs=1))
        state = ctx.enter_context(tc.tile_pool(name="state", bufs=1))
        tmps = ctx.enter_context(tc.tile_pool(name="tmps", bufs=2))
        smalls = ctx.enter_context(tc.tile_pool(name="smalls", bufs=1))
        psum = ctx.enter_context(tc.tile_pool(name="psum", bufs=2, space="PSUM"))

        # ---- resident weights: wT2 [(q,d), j, k], chunked loads ----
        w_sb = singles.tile([P, J, K], f32)
        WCH = 10
        for j0 in range(0, J, WCH):
            nc.sync.dma_start(
                out=w_sb[:, j0 : j0 + WCH, :], in_=wT2[:, j0 : j0 + WCH, :]
            )

        # per-b-tile state tiles (allocated once; tiles processed serially)
        u_sb = state.tile([P, J, P], f32)
        mask_sb = state.tile([P, S], f32)
        eb_sb = state.tile([P, D], f32)
        cw = state.tile([P, NI, S], f32)
        dl = state.tile([P, NI, S], f32)
        sw = state.tile([P, NI, S], f32)
        capa = state.tile([P, NI, D], f32)
        cap = state.tile([P, NI, D], f32)

        SBH = SB // 2

        def hat_batch(t, s0):
            """TensorE: hat for s in [s0, s0+SB) into PSUM.

            Matmuls are emitted parity-grouped (all even-s, then all odd-s)
            with parity-major PSUM layout: concurrent matmuls from different
            PE row-groups (base partition 0 vs 64) must not write the same
            PSUM bank -- interleaved emission hard-faults on HW.
            ps[p, q, i, k] holds s = s0 + 2*i + q.
            """
            ps = psum.tile([P, 2, SBH, K], f32, tag="ps")
            j0 = s0 // 2
            for q in (0, 1):
                for i in range(SBH):
                    nc.tensor.matmul(
                        ps[:, q, i, :],
                        u_sb[64 * q : 64 * q + 64, j0 + i, :],
                        w_sb[64 * q : 64 * q + 64, j0 + i, :],
                        start=True,
                        stop=True,
                    )
            return ps

        def e1_pass(t, sw_ap_fn, out_acc):
            """cap_acc[b,(n,d')] = sum_s sw[b,n,s] * hat[b,s,(n,d')]."""
            nc.vector.memset(out_acc[:], 0.0)
            for s0 in range(0, S, SB):
                ps = hat_batch(t, s0)
                tmp = tmps.tile([P, SBH, 2, NI, D], f32, tag="tmp")
                nc.vector.tensor_mul(
                    tmp[:],
                    ps[:].rearrange("p q i (n d) -> p i q n d", n=NI),
                    sw_ap_fn(s0),
                )
                red = smalls.tile([P, NI, D], f32, tag="red")
                nc.vector.tensor_reduce(
                    red[:],
                    tmp[:].transpose([0, 3, 4, 1, 2]),
                    axis=mybir.AxisListType.XY,
                    op=Alu.add,
                )
                nc.vector.tensor_add(out_acc[:], out_acc[:], red[:])

        def delta_pass(t, cap_in, out_delta):
            """delta[b,n,s] = sum_d' hat[b,s,(n,d')] * cap[b,n,d']."""
            for s0 in range(0, S, SB):
                ps = hat_batch(t, s0)
                tmp = tmps.tile([P, SBH, 2, NI, D], f32, tag="tmp")
                nc.vector.tensor_mul(
                    tmp[:],
                    ps[:].rearrange("p q i (n d) -> p i q n d", n=NI),
                    cap_in[:, None, None, :, :].broadcast_to([P, SBH, 2, NI, D]),
                )
                nc.vector.tensor_reduce(
                    out_delta[:, :, s0 : s0 + SB].rearrange(
                        "p n (i q) -> p i q n", q=2
                    ),
                    tmp[:],
                    axis=mybir.AxisListType.X,
                    op=Alu.add,
                )

        def squash(cin, cout):
            sq = smalls.tile([P, NI, D], f32, tag="sq")
            nc.vector.tensor_mul(sq[:], cin[:], cin[:])
            r2 = smalls.tile([P, NI], f32, tag="r2")
            nc.vector.tensor_reduce(
                r2[:], sq[:], axis=mybir.AxisListType.X, op=Alu.add
            )
            t1 = smalls.tile([P, NI], f32, tag="t1")
            nc.vector.tensor_scalar_add(t1[:], r2[:], 1.0)
            i1 = smalls.tile([P, NI], f32, tag="i1")
            nc.vector.reciprocal(i1[:], t1[:])
            r2b = smalls.tile([P, NI], f32, tag="r2b")
            nc.vector.tensor_scalar_add(r2b[:], r2[:], 1e-9)
            s1 = smalls.tile([P, NI], f32, tag="s1")
            nc.scalar.activation(s1[:], r2b[:], Act.Sqrt, bias=0.0, scale=1.0)
            i2 = smalls.tile([P, NI], f32, tag="i2")
            nc.vector.reciprocal(i2[:], s1[:])
            al = smalls.tile([P, NI], f32, tag="al")
            nc.vector.tensor_mul(al[:], r2[:], i1[:])
            nc.vector.tensor_mul(al[:], al[:], i2[:])
            nc.vector.tensor_mul(
                cout[:], cin[:], al[:, :, None].broadcast_to([P, NI, D])
            )

        def softmax_masked(cw_in, sw_out):
            mx = smalls.tile([P, S], f32, tag="mx")
            nc.vector.tensor_reduce(
                mx[:], cw_in[:].transpose([0, 2, 1]), axis=mybir.AxisListType.X,
                op=Alu.max,
            )
            nc.vector.tensor_sub(
                sw_out[:], cw_in[:], mx[:, None, :].broadcast_to([P, NI, S])
            )
            nc.scalar.activation(sw_out[:], sw_out[:], Act.Exp)
            z = smalls.tile([P, S], f32, tag="z")
            nc.vector.tensor_reduce(
                z[:], sw_out[:].transpose([0, 2, 1]), axis=mybir.AxisListType.X,
                op=Alu.add,
            )
            rz = smalls.tile([P, S], f32, tag="rz")
            nc.vector.reciprocal(rz[:], z[:])
            mz = smalls.tile([P, S], f32, tag="mz")
            nc.vector.tensor_mul(mz[:], mask_sb[:], rz[:])
            nc.vector.tensor_mul(
                sw_out[:], sw_out[:], mz[:, None, :].broadcast_to([P, NI, S])
            )

        for t in range(NT):
            # ---- loads for this b-tile (u slice for these 128 batch rows) ----
            UCH = 10
            for j0 in range(0, J, UCH):
                nc.sync.dma_start(
                    out=u_sb[:, j0 : j0 + UCH, :],
                    in_=uT2[:, j0 : j0 + UCH, t * P : t * P + P],
                )
            nc.sync.dma_start(out=mask_sb[:], in_=mask_d[t * P : t * P + P, :])
            nc.sync.dma_start(out=eb_sb[:], in_=eb_d[t * P : t * P + P, :])

            # ---- routing ----
            # iter 0: sw0 = 0.25*mask for every n (0.25 folded in post-sum)
            e1_pass(
                t,
                lambda s0: mask_sb[:, s0 : s0 + SB]
                .rearrange("p (i q) -> p i q", q=2)[:, :, :, None, None]
                .broadcast_to([P, SBH, 2, NI, D]),
                capa,
            )
            nc.vector.tensor_scalar_mul(capa[:], capa[:], 0.25)
            squash(capa, cap)
            delta_pass(t, cap, cw)  # cw(1) = delta0

            # iter 1
            softmax_masked(cw, sw)
            e1_pass(
                t,
                lambda s0: sw[:, :, s0 : s0 + SB]
                .rearrange("p n (i q) -> p i q n", q=2)[:, :, :, :, None]
                .broadcast_to([P, SBH, 2, NI, D]),
                capa,
            )
            squash(capa, cap)
            delta_pass(t, cap, dl)
            nc.vector.tensor_add(cw[:], cw[:], dl[:])

            # iter 2
            softmax_masked(cw, sw)
            e1_pass(
                t,
                lambda s0: sw[:, :, s0 : s0 + SB]
                .rearrange("p n (i q) -> p i q n", q=2)[:, :, :, :, None]
                .broadcast_to([P, SBH, 2, NI, D]),
                capa,
            )
            squash(capa, cap)

            # ---- hard readout ----
            pr = smalls.tile([P, NI, D], f32, tag="pr")
            nc.vector.tensor_mul(
                pr[:], cap[:], eb_sb[:, None, :].broadcast_to([P, NI, D])
            )
            dt = smalls.tile([P, NI], f32, tag="dt")
            nc.vector.tensor_reduce(
                dt[:], pr[:], axis=mybir.AxisListType.X, op=Alu.add
            )
            mx1 = smalls.tile([P, 1], f32, tag="mx1")
            nc.vector.tensor_reduce(
                mx1[:], dt[:], axis=mybir.AxisListType.X, op=Alu.max
            )
            g = smalls.tile([P, NI], f32, tag="g")
            nc.vector.tensor_tensor(
                g[:], dt[:], mx1[:].broadcast_to([P, NI]), op=Alu.is_ge
            )
            # first-match priority: sel_n = g_n * prod_{m<n}(1-g_m)
            notk = smalls.tile([P, 1], f32, tag="notk")
            sel = smalls.tile([P, NI], f32, tag="sel")
            nc.vector.tensor_copy(sel[:, 0:1], g[:, 0:1])
            nc.vector.tensor_scalar(
                notk[:], g[:, 0:1], -1.0, 1.0, Alu.mult, Alu.add
            )
            for n in range(1, NI):
                nc.vector.tensor_mul(sel[:, n : n + 1], g[:, n : n + 1], notk[:])
                if n < NI - 1:
                    t2 = smalls.tile([P, 1], f32, tag="t2")
                    nc.vector.tensor_scalar(
                        t2[:], sel[:, n : n + 1], -1.0, 1.0, Alu.mult, Alu.add
                    )
                    nc.vector.tensor_mul(notk[:], notk[:], t2[:])
            ro = smalls.tile([P, D], f32, tag="ro")
            nc.vector.tensor_scalar_mul(ro[:], cap[:, 0, :], sel[:, 0:1])
            for n in range(1, NI):
                nc.vector.scalar_tensor_tensor(
                    out=ro[:],
                    in0=cap[:, n, :],
                    scalar=sel[:, n : n + 1],
                    in1=ro[:],
                    op0=Alu.mult,
                    op1=Alu.add,
                )

            # ---- store ----
            nc.sync.dma_start(
                out=cap_d[t * P : t * P + P, :],
                in_=cap[:].rearrange("p n d -> p (n d)"),
            )
            nc.sync.dma_start(out=ro_d[t * P : t * P + P, :], in_=ro[:])

    nc.finalize()
    return nc


_NC_CACHE = None


def _get_nc():
    global _NC_CACHE
    if _NC_CACHE is None:
        _NC_CACHE = _build_bass()
    return _NC_CACHE


def _pack_inputs(item_his_emb, item_eb, mask, w):
    u = np.ascontiguousarray(np.asarray(item_his_emb, dtype=np.float32))
    eb = np.ascontiguousarray(np.asarray(item_eb, dtype=np.float32))
    mk = np.ascontiguousarray(np.asarray(mask, dtype=np.float32))
    ww = np.asarray(w, dtype=np.float32)[0]  # [S, K, D]
    # wT2[(q,d), j, k] with s = 2j+q
    wT2 = np.ascontiguousarray(
        ww.reshape(J, 2, K, D).transpose(1, 3, 0, 2).reshape(P, J, K)
    )
    in_maps = []
    for c in range(NCORES):
        ul = u[c * BL : (c + 1) * BL]  # [BL, S, D]
        uT2 = np.ascontiguousarray(
            ul.reshape(BL, J, 2, D).transpose(2, 3, 1, 0).reshape(P, J, BL)
        )
        in_maps.append(
            {
                "uT2": uT2,
                "wT2": wT2,
                "mask": np.ascontiguousarray(mk[c * BL : (c + 1) * BL]),
                "eb": np.ascontiguousarray(eb[c * BL : (c + 1) * BL]),
            }
        )
    return in_maps


def run(inputs, trace=False, **spmd_kwargs):
    from concourse.bass_utils import run_bass_kernel_spmd

    in_maps = _pack_inputs(**inputs)
    nc = _get_nc()
    res = run_bass_kernel_spmd(
        nc, in_maps, core_ids=list(range(NCORES)), trace=trace, **spmd_kwargs
    )
    caps = np.concatenate([res.results[c]["cap"] for c in range(NCORES)], axis=0)
    ros = np.concatenate([res.results[c]["ro"] for c in range(NCORES)], axis=0)
    return (caps.reshape(B, NI, D), ros), res


def kernel(**inputs):
    (caps, ros), _ = run(inputs, trace=False)
    return caps, ros


if __name__ == "__main__":
    rng = np.random.default_rng(0)
    ins = {
        "item_his_emb": rng.standard_normal((B, S, D), dtype=np.float32),
        "item_eb": rng.standard_normal((B, D), dtype=np.float32),
        "mask": rng.integers(0, 2, (B, S)).astype(np.float32),
        "w": rng.standard_normal((1, S, K, D), dtype=np.float32),
    }
    out, _ = run(ins)
    print([o.shape for o in out])
